# revision 21
# baseline (speedup 1.0000x reference)
"""Dense dot-product attention (B=4, H=16, S=2048, D=64) on 8 TRN2 NeuronCores.

Sharding: the 64 (b, h) slices are split 8-per-core (batch+head parallel, no
communication). Per slice, scores are computed transposed (S^T[k, q]) so the
softmax numerator exp(S^T) is already laid out as P^T for the P@V matmul:

  S^T chunk [128k, 512q] = matmul(lhsT=K^T[64d, 128k], rhs=Q^T[64d, 512q])
  P^T = exp(S^T)                      (ScalarE, PSUM -> SBUF)
  out'^T [65, 512q] += matmul(lhsT=V'[128k, 65], rhs=P^T[128k, 512q])

where V' = [V | ones] so row 64 of out'^T is the softmax denominator.
No max-subtraction: scores ~ N(0, 64), |s| < ~55, exp stays in fp32 range and
softmax is shift-invariant.

Engine balance per core: ScalarE exp work ~250us (33.5M exps at 1.2GHz x 128
lanes) vs PE matmul work ~237us -- ScalarE is the bottleneck, so the schedule
keeps it saturated:
 - Q/K transposes run pairwise ([128, 2x64] -> [128, 128] through the PE in
   f32r, 1.5 cyc/row), amortizing two chunks per identity stream; the odd
   chunk of each pair lands on partitions 64-127 and is moved back to 0-63
   by a small SBUF<-PSUM DMA while DVE copies the even half.
 - Work is emitted as a stream of "filler units" (PV chunks of the previous
   q-block, next-slice transpose groups, output epilogues) consumed against
   per-gap time budgets between QK groups, so the next QK group -- and hence
   the next exp -- is never delayed and ScalarE never drains at q-block or
   slice boundaries.
 - The first QK group of each q-block is a single chunk placed in the shared
   staging bank and pre-issued at the end of the previous q-block (that bank
   has no WAR against the main PSUM rings), so ScalarE rolls across q-block
   boundaries without waiting for a full QK group.

PSUM (8 banks): 4-bank ping (A: 3/4-chunk groups) + 2-bank pong (B: 2-chunk
groups) + out' accum (1) + shared staging/boundary bank (S, 1).
"""

import sys

sys.path.insert(0, "/opt/trn_rl_repo")

from collections import deque
from contextlib import ExitStack

import numpy as np

import bass_rust
import concourse.bass as bass
import concourse.tile as tile
from concourse import mybir
from concourse.bass_utils import run_bass_kernel_spmd
from concourse.masks import make_identity

B, H, S, D = 4, 16, 2048, 64
NCORES = 8
NS = (B * H) // NCORES  # slices per core
NCH = S // 128          # 16 key chunks per slice
NQB = S // 512          # 4 q-blocks per slice
F32 = mybir.dt.float32
F32R = mybir.dt.float32r
EXP = mybir.ActivationFunctionType.Exp
BF16 = mybir.dt.bfloat16

# QK chunk groups per q-block after the pre-issued 1-chunk boundary group:
# (start_chunk, n_chunks, ring) with ring A = 4-bank ping, B = 2-bank pong.
# 3-chunk A-groups keep the A-ring WAR cycle (exp -> next QK on the same
# banks, 2 groups later) comfortably shorter than ScalarE's per-q-block work,
# so the exp stream never structurally stalls on PSUM reuse.
QK_GROUPS = (
    (1, 2, "B"), (3, 3, "A"), (6, 2, "B"), (8, 3, "A"), (11, 2, "B"), (13, 3, "A")
)

# fill budgets (ns) for the gap after each main group; the last budget covers
# exp(last) + exp(next g0a) minus the pre-issued QK emits.
FILL_BUDGET = (348, 988, 348, 988, 348, 1337)
PV_NS = 213
TRP_NS = 80
OTR_NS = 41

_ENGINE_NS = {
    mybir.EngineType.SP: "sync",
    mybir.EngineType.PE: "tensor",
    mybir.EngineType.Activation: "scalar",
    mybir.EngineType.DVE: "vector",
    mybir.EngineType.Pool: "gpsimd",
}


def _fix_multiwait(nc):
    """This walrus build accepts only one sync wait per instruction. Tile can
    emit several; move extra waits onto preceding single-wait same-engine
    nops (queue stalls on the nop, same semantics)."""
    n_fixed = 0
    for f in nc.m.functions:
        for bb in f.blocks:
            il = bb.instructions
            for ins in list(il):
                si = ins.sync_info
                if si is None or ins.engine not in _ENGINE_NS:
                    continue
                waits = list(si.on_wait)
                if len(waits) <= 1:
                    continue
                ins.sync_info = bass_rust.SyncInfo(
                    on_wait=[waits[-1]], on_update=list(si.on_update)
                )
                eng = getattr(nc, _ENGINE_NS[ins.engine])
                idx = il.index(ins)
                for w in waits[:-1]:
                    nop_ins = eng.nop().ins
                    nop_ins.sync_info = bass_rust.SyncInfo(on_wait=[w], on_update=[])
                    for f2 in nc.m.functions:
                        for bb2 in f2.blocks:
                            il2 = bb2.instructions
                            for kk in range(len(il2) - 1, -1, -1):
                                if il2[kk] is nop_ins:
                                    del il2[kk]
                    il.insert(idx, nop_ins)
                    idx += 1
                n_fixed += 1
    return n_fixed


def _attention_body(ctx: ExitStack, tc: tile.TileContext, q, k, v, o):
    nc = tc.nc

    singles = ctx.enter_context(tc.tile_pool(name="singles", bufs=1))
    nat = ctx.enter_context(tc.tile_pool(name="nat", bufs=2))
    vpool = ctx.enter_context(tc.tile_pool(name="vpool", bufs=3))
    tpool = ctx.enter_context(tc.tile_pool(name="tpool", bufs=2))
    ptp = ctx.enter_context(tc.tile_pool(name="ptp", bufs=3))
    osb = ctx.enter_context(tc.tile_pool(name="osb", bufs=2))
    oout = ctx.enter_context(tc.tile_pool(name="oout", bufs=2))
    rp = ctx.enter_context(tc.tile_pool(name="rp", bufs=8))
    trtmp = ctx.enter_context(tc.tile_pool(name="trtmp", bufs=2))
    psA = ctx.enter_context(tc.tile_pool(name="psA", bufs=1, space="PSUM"))
    psB = ctx.enter_context(tc.tile_pool(name="psB", bufs=1, space="PSUM"))
    pso = ctx.enter_context(tc.tile_pool(name="pso", bufs=1, space="PSUM"))
    psmt = ctx.enter_context(tc.tile_pool(name="psmt", bufs=1, space="PSUM"))

    ident_f = singles.tile([128, 128], F32)
    make_identity(nc, ident_f)

    # per-slice / per-qb live state
    nat_t = {}   # s -> (q_nat, k_nat)
    vsb = {}     # s -> v_sb
    tts = {}     # s -> (qt, kt)
    pts = {}     # (s, qb) -> pt
    pos = {}     # (s, qb) -> po
    osbs = {}    # (s, qb) -> o_sb

    fillers = deque()  # (cost_ns, fn)

    def fill(budget):
        while fillers and budget > 0:
            c, fn = fillers.popleft()
            fn()
            budget -= c

    def drain_fillers():
        while fillers:
            _, fn = fillers.popleft()
            fn()

    def emit_load_qk(s):
        q_nat = nat.tile([128, NCH, 64], F32, tag="qnat")
        nc.sync.dma_start(out=q_nat, in_=q[s].rearrange("(n p) d -> p n d", p=128))
        k_nat = nat.tile([128, NCH, 64], F32, tag="knat")
        nc.sync.dma_start(out=k_nat, in_=k[s].rearrange("(n p) d -> p n d", p=128))
        nat_t[s] = (q_nat, k_nat)

    def emit_load_v(s):
        v_f32 = nat.tile([128, NCH, 65], F32, tag="vf32")
        nc.sync.dma_start(
            out=v_f32[:, :, 0:64], in_=v[s].rearrange("(n p) d -> p n d", p=128)
        )
        nc.gpsimd.memset(v_f32[:, :, 64:65], 1.0)
        v_sb = vpool.tile([128, NCH, 65], BF16, tag="vsb")
        nc.vector.tensor_copy(v_sb, v_f32)
        vsb[s] = v_sb

    def ensure_tt(s):
        if s not in tts:
            qt = tpool.tile([64, S], F32R, tag="qt")
            kt = tpool.tile([64, S], F32R, tag="kt")
            tts[s] = (qt, kt)
        return tts[s]

    def tr_unit(s, g):
        """One stg group: 4 pair-transposes = 8 chunks of Q (g<2) or K.

        Pair (c, c+1) transposes [128, 128] -> [128, 128]: chunk c lands on
        partitions 0-63, chunk c+1 on 64-127. DVE copies the even chunks to
        qt/kt; a small DMA moves the odd chunks' partitions back down.
        """
        qt, kt = ensure_tt(s)
        src, dst = (nat_t[s][0], qt) if g < 2 else (nat_t[s][1], kt)
        h = g % 2  # which half of the chunks (0-7 or 8-15)
        def unit():
            stg = psmt.tile([128, 512], F32, tag="stg")
            for j in range(4):
                c = 8 * h + 2 * j
                nc.tensor.transpose(
                    out=stg[:, j * 128 : (j + 1) * 128],
                    in_=src[:, c : c + 2, :],
                    identity=ident_f,
                )
            blk = dst[0:64, 8 * h * 128 : (8 * h + 8) * 128].rearrange(
                "p (pr two f) -> p pr two f", two=2, f=128
            )
            stg_v = stg.rearrange("p (pr f) -> p pr f", f=128)
            # tensor_copy f32 -> f32r performs the f32r rounding the QK
            # matmuls require of their inputs
            nc.vector.tensor_copy(blk[:, :, 0, :], stg_v[0:64])
            # odd chunks sit on partitions 64-127: DVE is lane-locked, so
            # stage them to SBUF and DMA the partitions back down to 0-63
            tmp = trtmp.tile([128, 512], F32R, tag="trtmp", name="trtmp")
            tmp_v = tmp.rearrange("p (pr f) -> p pr f", f=128)
            nc.vector.tensor_copy(tmp_v[64:128], stg_v[64:128])
            nc.sync.dma_start(out=blk[:, :, 1, :], in_=tmp_v[64:128])
        return (4 * TRP_NS, unit)

    def pv_unit(s, qb, c):
        v_sb = vsb[s]
        pt = pts[(s, qb)]
        def unit():
            if c == 0:
                pos[(s, qb)] = pso.tile([65, 512], F32, tag="po", name="po")
            nc.tensor.matmul(
                out=pos[(s, qb)][:],
                lhsT=v_sb[:, c, :],
                rhs=pt[:, c * 512 : (c + 1) * 512],
                start=(c == 0),
                stop=(c == NCH - 1),
            )
            if c == NCH - 1:
                # numerator+denominator accumulated; stage to SBUF (DVE)
                o_sb = osb.tile([65, 512], F32, tag="osb")
                nc.vector.tensor_copy(o_sb, pos[(s, qb)])
                osbs[(s, qb)] = o_sb
                del pos[(s, qb)], pts[(s, qb)]
        return (PV_NS, unit)

    def epi_unit(s, qb):
        """Out-transpose + divide + DMA for (s, qb); o_sb staged earlier."""
        def unit():
            o_sb = osbs.pop((s, qb))
            ot = psmt.tile([128, 512], F32, tag="stg")
            for i in range(4):
                nc.tensor.transpose(
                    out=ot[:, i * 65 : (i + 1) * 65],
                    in_=o_sb[:, i * 128 : (i + 1) * 128],
                    identity=ident_f[0:65, 0:65],
                )
            o_out = oout.tile([128, 4, 64], F32, tag="oout")
            for i in range(4):
                r = rp.tile([128, 1], F32, tag="r")
                nc.vector.reciprocal(r, ot[:, i * 65 + 64 : i * 65 + 65])
                nc.vector.tensor_scalar_mul(
                    o_out[:, i, :], ot[:, i * 65 : i * 65 + 64], r
                )
            o_re = o[s].rearrange("(n p) d -> p n d", p=128)
            nc.sync.dma_start(out=o_re[:, qb * 4 : (qb + 1) * 4, :], in_=o_out)
        return (4 * OTR_NS, unit)

    def pre_issue(s, qb):
        """Boundary group (chunk 0) of (s, qb): QK into the staging bank +
        exp, emitted at the end of the previous q-block."""
        qt, kt = tts[s]
        pts[(s, qb)] = ptp.tile([128, NCH * 512], BF16, tag="pt", name="pt")
        ps = psmt.tile([128, 512], F32, tag="stg")
        nc.tensor.matmul(
            out=ps[:],
            lhsT=kt[0:64, 0:128],
            rhs=qt[0:64, qb * 512 : (qb + 1) * 512],
            start=True,
            stop=True,
        )
        nc.scalar.activation(
            out=pts[(s, qb)][:, 0:512], in_=ps[:], func=EXP
        )

    def emit_qk_group(s, qb, gi):
        c0, n, ring = QK_GROUPS[gi]
        qt, kt = tts[s]
        pt = pts[(s, qb)]
        pool = psA if ring == "A" else psB
        width = 4 if ring == "A" else 2
        ps = pool.tile([128, width * 512], F32, tag=f"sg{ring}", name="ps")
        for j in range(n):
            c = c0 + j
            nc.tensor.matmul(
                out=ps[:, j * 512 : (j + 1) * 512],
                lhsT=kt[0:64, c * 128 : (c + 1) * 128],
                rhs=qt[0:64, qb * 512 : (qb + 1) * 512],
                start=True,
                stop=True,
            )
        nc.scalar.activation(
            out=pt[:, c0 * 512 : (c0 + n) * 512],
            in_=ps[:, 0 : n * 512],
            func=EXP,
        )

    def push_qb_fillers(s, qb):
        """Fillers consumed during (s, qb): PV c4-15 of the previous q-block
        plus c0-3 of the current one (so no unit queued near the q-block
        boundary waits on the exp that also gates the next QK group), one
        transpose group for slice s+1, the epilogue two q-blocks back."""
        if (s, qb) == (0, 0):
            pv = None
        elif qb == 0:
            pv = (s - 1, NQB - 1)
        else:
            pv = (s, qb - 1)
        if pv is None or pv == (0, 0):
            epi = None
        elif pv[1] == 0:
            epi = (pv[0] - 1, NQB - 1) if pv[0] > 0 else None
        else:
            epi = (pv[0], pv[1] - 1)
        tr = tr_unit(s + 1, qb) if s + 1 < NS else None

        units = []
        if pv is not None:
            units += [pv_unit(*pv, c) for c in range(4, 8)]
        if tr is not None:
            units.append(tr)
        if pv is not None:
            units += [pv_unit(*pv, c) for c in range(8, 12)]
        if epi is not None:
            units.append(epi_unit(*epi))
        if pv is not None:
            units += [pv_unit(*pv, c) for c in range(12, NCH)]
        units += [pv_unit(s, qb, c) for c in range(0, 4)]
        if (s, qb) == (NS - 1, NQB - 1):
            # final q-block: nothing follows, so consume its own PV chunks
            # in its gaps as their exps complete
            units += [pv_unit(s, qb, c) for c in range(4, 12)]
        fillers.extend(units)

    # ---- prologue. Slice 0 is latency-critical: load k/q in halves (k
    # first) and transpose with single-chunk transposes (no cross-partition
    # odd-chunk fix-up on the critical path) so the first exp can start as
    # soon as kt chunk 0 and qt chunks 0-3 exist.
    q_nat = nat.tile([128, NCH, 64], F32, tag="qnat")
    k_nat = nat.tile([128, NCH, 64], F32, tag="knat")
    q_re = q[0].rearrange("(n p) d -> p n d", p=128)
    k_re = k[0].rearrange("(n p) d -> p n d", p=128)
    nc.sync.dma_start(out=k_nat[:, 0:8, :], in_=k_re[:, 0:8, :])
    nc.sync.dma_start(out=q_nat[:, 0:8, :], in_=q_re[:, 0:8, :])
    nat_t[0] = (q_nat, k_nat)

    def tr_single(src, dst, c0):
        stg = psmt.tile([128, 512], F32, tag="stg", name="stg")
        for j in range(4):
            nc.tensor.transpose(
                out=stg[0:64, j * 128 : (j + 1) * 128],
                in_=src[:, c0 + j, :],
                identity=ident_f,
            )
        nc.vector.tensor_copy(
            dst[0:64, c0 * 128 : (c0 + 4) * 128], stg[0:64, :]
        )

    qt0, kt0 = ensure_tt(0)
    tr_single(k_nat, kt0, 0)
    tr_single(q_nat, qt0, 0)
    pre_issue(0, 0)
    tr_single(k_nat, kt0, 4)
    tr_single(q_nat, qt0, 4)
    nc.sync.dma_start(out=k_nat[:, 8:16, :], in_=k_re[:, 8:16, :])
    nc.sync.dma_start(out=q_nat[:, 8:16, :], in_=q_re[:, 8:16, :])
    emit_load_v(0)
    tr_single(k_nat, kt0, 8)
    tr_single(q_nat, qt0, 8)
    tr_single(k_nat, kt0, 12)
    tr_single(q_nat, qt0, 12)
    emit_load_qk(1)

    for s in range(NS):
        for qb in range(NQB):
            if qb == 1 and s + 1 < NS:
                emit_load_v(s + 1)
            if qb == 2 and s + 2 < NS:
                emit_load_qk(s + 2)
            push_qb_fillers(s, qb)
            for gi in range(len(QK_GROUPS)):
                emit_qk_group(s, qb, gi)
                if gi < len(QK_GROUPS) - 1:
                    fill(FILL_BUDGET[gi])
            # pre-issue the next q-block's boundary group, then fill the
            # remaining exp(g5)+exp(g0a) shadow
            if qb + 1 < NQB:
                pre_issue(s, qb + 1)
            elif s + 1 < NS:
                pre_issue(s + 1, 0)
            fill(FILL_BUDGET[-1])

    # ---- drain: remaining PV + epilogues ----
    drain_fillers()
    s_last = NS - 1
    fillers.extend(pv_unit(s_last, NQB - 1, c) for c in range(12, NCH))
    fillers.append(epi_unit(s_last, NQB - 2))
    drain_fillers()
    fillers.append(epi_unit(s_last, NQB - 1))
    drain_fillers()


def _build(loop_r=None):
    nc = bass.Bass(num_devices=NCORES)
    q = nc.dram_tensor("q", [NS, S, D], F32, kind="ExternalInput")
    k = nc.dram_tensor("k", [NS, S, D], F32, kind="ExternalInput")
    v = nc.dram_tensor("v", [NS, S, D], F32, kind="ExternalInput")
    o = nc.dram_tensor("o", [NS, S, D], F32, kind="ExternalOutput")
    with tile.TileContext(nc) as tc:
        with ExitStack() as ctx:
            if loop_r:
                with tc.For_i(0, loop_r, 1):
                    _attention_body(ctx, tc, q.ap(), k.ap(), v.ap(), o.ap())
            else:
                _attention_body(ctx, tc, q.ap(), k.ap(), v.ap(), o.ap())
    _fix_multiwait(nc)
    return nc


def kernel(Q, K, V, _trace=False, _trace_kwargs=None):
    Qr = np.ascontiguousarray(Q.reshape(NCORES, NS, S, D))
    Kr = np.ascontiguousarray(K.reshape(NCORES, NS, S, D))
    Vr = np.ascontiguousarray(V.reshape(NCORES, NS, S, D))
    nc = _build()
    in_maps = [
        {"q": Qr[i], "k": Kr[i], "v": Vr[i]} for i in range(NCORES)
    ]
    res = run_bass_kernel_spmd(
        nc, in_maps, core_ids=list(range(NCORES)), trace=_trace,
        **(_trace_kwargs or {}),
    )
    out = np.stack([res.results[i]["o"] for i in range(NCORES)], axis=0)
    out = out.reshape(B, H, S, D).astype(np.float32, copy=False)
    if _trace:
        return out, res
    return out


# revision 22
# speedup vs baseline: 1.0686x; 1.0686x over previous
"""Dense dot-product attention (B=4, H=16, S=2048, D=64) on 8 TRN2 NeuronCores.

Sharding: the 64 (b, h) slices are split 8-per-core (batch+head parallel, no
communication). Per slice, scores are computed transposed (S^T[k, q]) so the
softmax numerator exp(S^T) is already laid out as P^T for the P@V matmul:

  S^T chunk [128k, 512q] = matmul(lhsT=K^T[64d, 128k], rhs=Q^T[64d, 512q])
  P^T = exp(S^T)                      (ScalarE, PSUM -> SBUF)
  out'^T [65, 512q] += matmul(lhsT=V'[128k, 65], rhs=P^T[128k, 512q])

where V' = [V | ones] so row 64 of out'^T is the softmax denominator.
No max-subtraction: scores ~ N(0, 64), |s| < ~55, exp stays in fp32 range and
softmax is shift-invariant. Final transpose back to [q, d] on the PE, divide
by the denominator on VectorE, DMA out.

QK matmuls run in float32r (fast fp32 PE path; fp32 proper is 4 cyc/row);
the exp writes P^T in bf16 and V' is bf16, so the P@V side streams bf16.
PV of q-block i is interleaved into the QK-group gaps of block i+1 so the
in-order PE stays busy while QK waits on exp's PSUM WAR (4/2/4/2/4-bank
ping-pong + out' + transpose-staging = 8 PSUM banks).
"""

import sys

sys.path.insert(0, "/opt/trn_rl_repo")

from contextlib import ExitStack

import numpy as np

import bass_rust
import concourse.bass as bass
import concourse.tile as tile
from concourse import mybir
from concourse.bass_utils import run_bass_kernel_spmd
from concourse.masks import make_identity

B, H, S, D = 4, 16, 2048, 64
NCORES = 8
NS = (B * H) // NCORES  # slices per core
NCH = S // 128          # 16 key chunks per slice
NQB = S // 512          # 4 q-blocks per slice
F32 = mybir.dt.float32
F32R = mybir.dt.float32r
EXP = mybir.ActivationFunctionType.Exp
BF16 = mybir.dt.bfloat16

# QK chunk groups per q-block: (start_chunk, n_chunks). Sized so the PSUM
# ping-pong (4-bank + 2-bank) plus out' (1) and transpose staging (1) fit in
# the 8 PSUM banks while ScalarE reads big (2048/1024-elem) spans.
QK_GROUPS = ((0, 4), (4, 2), (6, 4), (10, 2), (12, 4))


_ENGINE_NS = {
    mybir.EngineType.SP: "sync",
    mybir.EngineType.PE: "tensor",
    mybir.EngineType.Activation: "scalar",
    mybir.EngineType.DVE: "vector",
    mybir.EngineType.Pool: "gpsimd",
}


def _fix_multiwait(nc):
    """This walrus build accepts only one sync wait per instruction. Tile can
    emit several; move extra waits onto preceding single-wait same-engine
    nops (queue stalls on the nop, same semantics)."""
    n_fixed = 0
    for f in nc.m.functions:
        for bb in f.blocks:
            il = bb.instructions
            for ins in list(il):
                si = ins.sync_info
                if si is None or ins.engine not in _ENGINE_NS:
                    continue
                waits = list(si.on_wait)
                if len(waits) <= 1:
                    continue
                ins.sync_info = bass_rust.SyncInfo(
                    on_wait=[waits[-1]], on_update=list(si.on_update)
                )
                eng = getattr(nc, _ENGINE_NS[ins.engine])
                idx = il.index(ins)
                for w in waits[:-1]:
                    nop_ins = eng.nop().ins
                    nop_ins.sync_info = bass_rust.SyncInfo(on_wait=[w], on_update=[])
                    for f2 in nc.m.functions:
                        for bb2 in f2.blocks:
                            il2 = bb2.instructions
                            for kk in range(len(il2) - 1, -1, -1):
                                if il2[kk] is nop_ins:
                                    del il2[kk]
                    il.insert(idx, nop_ins)
                    idx += 1
                n_fixed += 1
    return n_fixed


def _attention_body(ctx: ExitStack, tc: tile.TileContext, q, k, v, o, dup=()):
    nc = tc.nc

    singles = ctx.enter_context(tc.tile_pool(name="singles", bufs=1))
    nat = ctx.enter_context(tc.tile_pool(name="nat", bufs=2))
    vpool = ctx.enter_context(tc.tile_pool(name="vpool", bufs=2))
    tpool = ctx.enter_context(tc.tile_pool(name="tpool", bufs=2))
    ptp = ctx.enter_context(tc.tile_pool(name="ptp", bufs=2))
    osb = ctx.enter_context(tc.tile_pool(name="osb", bufs=2))
    oout = ctx.enter_context(tc.tile_pool(name="oout", bufs=2))
    rp = ctx.enter_context(tc.tile_pool(name="rp", bufs=8))
    ps4 = ctx.enter_context(tc.tile_pool(name="ps4", bufs=1, space="PSUM"))
    ps2 = ctx.enter_context(tc.tile_pool(name="ps2", bufs=1, space="PSUM"))
    pso = ctx.enter_context(tc.tile_pool(name="pso", bufs=1, space="PSUM"))
    psmt = ctx.enter_context(tc.tile_pool(name="psmt", bufs=1, space="PSUM"))

    ident = singles.tile([128, 128], F32)
    make_identity(nc, ident)

    # software pipeline: PV + epilogue of q-block i is interleaved between the
    # QK groups of q-block i+1 so the PE has queued work while QK waits on the
    # exp (PSUM WAR) of its own block. state: [v_sb, pt, s, qb, po, next_chunk]
    pending = []

    def emit_pv(nchunks):
        if not pending:
            return
        st = pending[0]
        v_sb, pt, s, qb, po, c0 = st
        if po is None:
            po = pso.tile([65, 512], F32, tag="po")
            st[4] = po
        reps = 2 if "pv" in dup else 1
        hi = min(c0 + nchunks, NCH * reps)
        for ci in range(c0, hi):
            c = ci % NCH
            nc.tensor.matmul(
                out=po[:],
                lhsT=v_sb[:, c, :],
                rhs=pt[:, c * 512 : (c + 1) * 512],
                start=(c == 0),
                stop=(c == NCH - 1),
            )
        st[5] = hi
        if hi < NCH * reps:
            return
        o_sb = osb.tile([65, 512], F32)
        nc.vector.tensor_copy(o_sb, po)
        ot = psmt.tile([128, 4 * 65], F32, tag="mt")
        for i in range(4):
            nc.tensor.transpose(
                out=ot[:, i * 65 : (i + 1) * 65],
                in_=o_sb[:, i * 128 : (i + 1) * 128],
                identity=ident[0:65, 0:65],
            )
        o_out = oout.tile([128, 4, 64], F32)
        for i in range(4):
            r = rp.tile([128, 1], F32)
            nc.vector.reciprocal(r, ot[:, i * 65 + 64 : i * 65 + 65])
            nc.vector.tensor_scalar_mul(
                o_out[:, i, :], ot[:, i * 65 : i * 65 + 64], r
            )
        o_re = o[s].rearrange("(n p) d -> p n d", p=128)
        nc.sync.dma_start(out=o_re[:, qb * 4 : (qb + 1) * 4, :], in_=o_out)
        pending.clear()

    def flush_pending():
        while pending:
            emit_pv(NCH)

    for s in range(NS):
        q_nat = nat.tile([128, NCH, 64], F32, tag="qnat")
        nc.sync.dma_start(out=q_nat, in_=q[s].rearrange("(n p) d -> p n d", p=128))
        k_nat = nat.tile([128, NCH, 64], F32, tag="knat")
        nc.sync.dma_start(out=k_nat, in_=k[s].rearrange("(n p) d -> p n d", p=128))
        v_f32 = nat.tile([128, NCH, 65], F32, tag="vf32")
        nc.sync.dma_start(
            out=v_f32[:, :, 0:64], in_=v[s].rearrange("(n p) d -> p n d", p=128)
        )
        nc.vector.memset(v_f32[:, :, 64:65], 1.0)
        v_sb = vpool.tile([128, NCH, 65], BF16)
        nc.vector.tensor_copy(v_sb, v_f32)

        qt = tpool.tile([64, S], F32R, tag="qt")
        kt = tpool.tile([64, S], F32R, tag="kt")
        for nat_t, tt in ((q_nat, qt), (k_nat, kt)):
            for g in range(4):
                stg = psmt.tile([64, 512], F32, tag="mt")
                for j in range(4):
                    c = 4 * g + j
                    for _rep in range(2 if "tr" in dup else 1):
                        nc.tensor.transpose(
                            out=stg[:, j * 128 : (j + 1) * 128],
                            in_=nat_t[:, c, :],
                            identity=ident,
                        )
                nc.vector.tensor_copy(tt[0:64, g * 512 : (g + 1) * 512], stg)

        for qb in range(NQB):
            pt = ptp.tile([128, NCH * 512], BF16)
            reps = 2 if "pv" in dup else 1
            pv_per_gap = (NCH * reps) // 5
            for c0, nch in QK_GROUPS:
                emit_pv(pv_per_gap)
                ps = (ps4 if nch == 4 else ps2).tile(
                    [128, nch * 512], F32, tag=f"sg{nch}"
                )
                for j in range(nch):
                    c = c0 + j
                    for _rep in range(2 if "qk" in dup else 1):
                        nc.tensor.matmul(
                            out=ps[:, j * 512 : (j + 1) * 512],
                            lhsT=kt[0:64, c * 128 : (c + 1) * 128],
                            rhs=qt[0:64, qb * 512 : (qb + 1) * 512],
                            start=True,
                            stop=True,
                        )
                for _rep in range(2 if "exp" in dup else 1):
                    nc.scalar.activation(
                        out=pt[:, c0 * 512 : (c0 + nch) * 512], in_=ps[:, :], func=EXP
                    )
            flush_pending()
            pending.append([v_sb, pt, s, qb, None, 0])
    flush_pending()


def _build(loop_r=None, dup=()):
    nc = bass.Bass(num_devices=NCORES)
    q = nc.dram_tensor("q", [NS, S, D], F32, kind="ExternalInput")
    k = nc.dram_tensor("k", [NS, S, D], F32, kind="ExternalInput")
    v = nc.dram_tensor("v", [NS, S, D], F32, kind="ExternalInput")
    o = nc.dram_tensor("o", [NS, S, D], F32, kind="ExternalOutput")
    with tile.TileContext(nc) as tc:
        with ExitStack() as ctx:
            if loop_r:
                with tc.For_i(0, loop_r, 1):
                    _attention_body(ctx, tc, q.ap(), k.ap(), v.ap(), o.ap(), dup)
            else:
                _attention_body(ctx, tc, q.ap(), k.ap(), v.ap(), o.ap(), dup)
    _fix_multiwait(nc)
    return nc


def kernel(Q, K, V, _trace=False, _trace_kwargs=None):
    Qr = np.ascontiguousarray(Q.reshape(NCORES, NS, S, D))
    Kr = np.ascontiguousarray(K.reshape(NCORES, NS, S, D))
    Vr = np.ascontiguousarray(V.reshape(NCORES, NS, S, D))
    nc = _build()
    in_maps = [
        {"q": Qr[i], "k": Kr[i], "v": Vr[i]} for i in range(NCORES)
    ]
    res = run_bass_kernel_spmd(
        nc, in_maps, core_ids=list(range(NCORES)), trace=_trace,
        **(_trace_kwargs or {}),
    )
    out = np.stack([res.results[i]["o"] for i in range(NCORES)], axis=0)
    out = out.reshape(B, H, S, D).astype(np.float32, copy=False)
    if _trace:
        return out, res
    return out

